# revision 18
# baseline (speedup 1.0000x reference)
"""Trainium2 Bass kernel for nn_BasicLSTM (B=64, T=512, D=512, U=1024).

Data-parallel over batch across 8 NeuronCores (8 sequences/core, recurrence
local per core).  v2 redesign over the reference kernel:

- x@Wx+b is hoisted out of the sequential loop: computed 16 steps at a time
  as full-M (128-token) matmuls into PSUM slabs, copied to an SBUF window
  buffer (fp16).  Per-step the window row block for step t is added into the
  z PSUM bank by a single K=128 matmul against a shifted-identity stationary
  operand (the "inject").  This removes the per-step bias openers and the
  M=8 x-matmuls of the v1 kernel (~2.1us/step -> ~0.65us/step of PE time).
- fp16 everywhere (weights, activations, cell state): DVE tensor_tensor ops
  run in 2x packed mode (fp32 runs 1x), and fp16's 10-bit mantissa more than
  halves the numerical error vs the bf16 v1 kernel.
- no explicit vector drains; WAR/RAW hazards are sequenced with semaphores
  and engine program order only.

Layout (per 512-col "bank", 2 banks = 1024 units): gate g of units in the
bank occupies PSUM partitions 32g..32g+8, weight columns host-permuted to
[i|f|o|g] blocks with g columns pre-scaled by 2 (tanh(x) = 2*sigmoid(2x)-1).
"""

import numpy as np

B, T, D, U = 64, 512, 512, 1024
NCORES = 8
BL = B // NCORES          # 8 sequences per core
NKX = D // 128            # 4 x K-chunks
NKH = U // 128            # 8 h K-chunks
NT = 512                  # bank width (one PSUM bank)
WSTEP = 16                # steps per xz window (= 128 tokens)
NSLAB = 8                 # 512-col slabs per window (4096 cols)
NWIN = T // WSTEP         # 32 windows
GOFF = (0, 32, 64, 96)    # PSUM partition offset per gate (i,f,o,g)


def _build_nc(t_steps=T):
    import concourse.bass as bass
    import concourse.mybir as mybir

    f32, f16 = mybir.dt.float32, mybir.dt.float16
    AF = mybir.ActivationFunctionType
    ALU = mybir.AluOpType

    nwin = t_steps // WSTEP
    nc = bass.Bass(num_devices=NCORES)
    wq = nc.declare_dram_parameter("wq", [1537, 4096], f16, isOutput=False)
    xq = nc.declare_dram_parameter("xq", [NKX, 128, t_steps * BL], f16, isOutput=False)
    eq = nc.declare_dram_parameter("eq", [128, WSTEP * BL], f16, isOutput=False)
    mq = nc.declare_dram_parameter("mq", [BL + 1, 128], f16, isOutput=False)
    zz = nc.declare_dram_parameter("zz", [128, 1024], f16, isOutput=False)
    out_d = nc.declare_dram_parameter("out", [BL, U], f32, isOutput=True)

    from contextlib import ExitStack
    ctx = ExitStack()
    sb = lambda shape, dt, name: ctx.enter_context(nc.sbuf_tensor(name, shape, dt))
    ps = lambda shape, dt, name: ctx.enter_context(nc.psum_tensor(name, shape, dt))
    sem = lambda name: ctx.enter_context(nc.semaphore(name))

    with ctx:
        wh_sb = sb([128, NKH * 4096], f16, "wh_sb")
        wx_sb = sb([128, NKX * 4096], f16, "wx_sb")
        xT_sb = sb([128, NKX * (t_steps * BL)], f16, "xT_sb")
        bias_sb = sb([1, 4096], f16, "bias_sb")
        ones_sb = sb([1, 128], f16, "ones_sb")
        E_sb = sb([128, WSTEP * BL], f16, "E_sb")
        ident = sb([BL, BL], f16, "ident")
        XZw = sb([128, 2 * 4096], f16, "XZw")
        s_sb = sb([128, 1024], f16, "s_sb")
        g_sb = sb([BL, 1024], f16, "g_sb")
        c_sb = sb([32 + BL, 1024], f16, "c_sb")
        c1_sb = sb([32 + BL, 1024], f16, "c1_sb")
        t1_sb = sb([32 + BL, 1024], f16, "t1_sb")
        tc_sb = sb([64 + BL, 1024], f16, "tc_sb")
        h_sb = sb([BL, 1024], f16, "h_sb")
        hT_sb = sb([128, 2 * NKH * BL], f16, "hT_sb")   # parity x (8 chunks x 8)
        hlast = sb([BL, 1024], f32, "hlast")

        zA = ps([128, NT], f32, "zA")
        zB = ps([128, NT], f32, "zB")
        trbuf = ps([128, 2 * 4 * BL], f16, "trbuf")      # A cols 0:32, B 32:64
        slab_ps = [ps([128, NT], f32, f"slab{i}") for i in range(2)]

        dma_sem = sem("dma_sem")
        mm_sem = sem("mm_sem")      # PE: z bank complete
        sig_sem = sem("sig_sem")    # scalar: sigmoid done
        zfree_sem = sem("zfree_sem")  # scalar: tanh_g done (all z bank reads done)
        csum_sem = sem("csum_sem")  # vector: c updated
        tanh_sem = sem("tanh_sem")  # scalar: tanh(c) done
        h_sem = sem("h_sem")        # vector: h bank done
        tr_sem = sem("tr_sem")      # PE: transposes done
        cp_sem = sem("cp_sem")      # scalar: hT copy done
        slab_sem = sem("slab_sem")  # PE: xz slab matmuls done
        xzc_sem = sem("xzc_sem")    # scalar: xz slab copied to SBUF
        st_sem = sem("st_sem")      # final store

        NDMA = NKH + 2 * NKX + 6  # wh(8), wx(4), xT(4), bias, E, ident, ones, hT0, c0

        def slab_mms(tensor, v, s, inc):
            """xz slab (window v, slab s): bias row + 4 x K-chunks, M=128."""
            p = slab_ps[s % 2]
            tensor.matmul(
                p[:, :], ones_sb[0:1, :], bias_sb[0:1, s * NT:(s + 1) * NT],
                start=True, stop=False, skip_group_check=True,
            )
            for kc in range(NKX):
                ins = tensor.matmul(
                    p[:, :],
                    xT_sb[:, kc * (t_steps * BL) + v * 128:
                          kc * (t_steps * BL) + v * 128 + 128],
                    wx_sb[:, kc * 4096 + s * NT:kc * 4096 + (s + 1) * NT],
                    start=False, stop=(kc == NKX - 1), skip_group_check=True,
                )
            ins.then_inc(slab_sem, 1)

        with nc.Block() as block:

            @block.sync
            def _(sync):
                for kc in range(NKH):
                    sync.dma_start(out=wh_sb[:, kc * 4096:(kc + 1) * 4096],
                                   in_=wq[512 + kc * 128:512 + (kc + 1) * 128, :]
                                   ).then_inc(dma_sem, 16)
                for kc in range(NKX):
                    sync.dma_start(out=wx_sb[:, kc * 4096:(kc + 1) * 4096],
                                   in_=wq[kc * 128:(kc + 1) * 128, :]
                                   ).then_inc(dma_sem, 16)
                for kc in range(NKX):
                    sync.dma_start(out=xT_sb[:, kc * (t_steps * BL):(kc + 1) * (t_steps * BL)],
                                   in_=xq[kc]).then_inc(dma_sem, 16)
                sync.dma_start(out=bias_sb[:, :], in_=wq[1536:1537, :]).then_inc(dma_sem, 16)
                sync.dma_start(out=E_sb[:, :], in_=eq[:, :]).then_inc(dma_sem, 16)
                sync.dma_start(out=ident[:, :], in_=mq[0:BL, 0:BL]).then_inc(dma_sem, 16)
                sync.dma_start(out=ones_sb[:, :], in_=mq[BL:BL + 1, :]).then_inc(dma_sem, 16)
                sync.dma_start(out=hT_sb[:, :], in_=zz[:, 0:2 * NKH * BL]).then_inc(dma_sem, 16)
                sync.dma_start(out=c_sb[32:32 + BL, :], in_=zz[0:BL, :]).then_inc(dma_sem, 16)
                # final store
                sync.wait_ge(h_sem, 2 * t_steps)
                sync.dma_start(out=out_d[:, :], in_=hlast[:, :]).then_inc(st_sem, 16)

            @block.tensor
            def _(tensor):
                tensor.wait_ge(dma_sem, 16 * NDMA)
                # one-time clear of z banks (stale PSUM may hold non-finite
                # garbage in the never-written partition rows)
                for zb in (zA, zB):
                    tensor.matmul(zb[:, :], hT_sb[:, 0:128], wh_sb[:, 0:NT],
                                  start=True, stop=True, skip_group_check=True)
                # window 0 precompute (slabs drain to scalar as they finish)
                for s in range(NSLAB):
                    if s >= 2:
                        tensor.wait_ge(xzc_sem, s - 1)
                    slab_mms(tensor, 0, s, True)

                for t in range(t_steps):
                    w, u = t // WSTEP, t % WSTEP
                    rp = (t - 1) % 2  # hT parity read this step
                    # transposes of h(t-1) bank A
                    if t >= 1:
                        tensor.wait_ge(h_sem, 2 * (t - 1) + 1)
                        if t >= 2:
                            tensor.wait_ge(cp_sem, 2 * (t - 2) + 1)
                        for j in range(4):
                            ins = tensor.matmul(
                                trbuf[:, j * BL:(j + 1) * BL],
                                h_sb[0:BL, j * 128:(j + 1) * 128],
                                ident[:, :], start=True, stop=True,
                                is_transpose=True, skip_group_check=True,
                            )
                        ins.then_inc(tr_sem, 1)
                    # bank A z
                    if t >= 1:
                        tensor.wait_ge(zfree_sem, 2 * (t - 1) + 1)
                    tensor.wait_ge(xzc_sem, NSLAB * (w + 1))
                    for g in range(4):
                        tensor.matmul(
                            zA[GOFF[g]:GOFF[g] + BL, :],
                            E_sb[:, u * BL:(u + 1) * BL],
                            XZw[:, (w % 2) * 4096 + g * NT:(w % 2) * 4096 + (g + 1) * NT],
                            start=True, stop=False,
                            tile_position=(0, GOFF[g]), skip_group_check=True,
                        )
                    for j in range(NKH):
                        if t >= 1 and j == 0:
                            tensor.wait_ge(cp_sem, 2 * (t - 1) + 1)
                        if j == 4:
                            # transposes of h(t-1) bank B (must precede the
                            # cp wait below: copyB depends on them)
                            if t >= 1:
                                tensor.wait_ge(h_sem, 2 * (t - 1) + 2)
                                if t >= 2:
                                    tensor.wait_ge(cp_sem, 2 * (t - 2) + 2)
                                for jj in range(4):
                                    ins = tensor.matmul(
                                        trbuf[:, 32 + jj * BL:32 + (jj + 1) * BL],
                                        h_sb[0:BL, 512 + jj * 128:512 + (jj + 1) * 128],
                                        ident[:, :], start=True, stop=True,
                                        is_transpose=True, skip_group_check=True,
                                    )
                                ins.then_inc(tr_sem, 1)
                                tensor.wait_ge(cp_sem, 2 * (t - 1) + 2)
                        lhsT = hT_sb[:, rp * (NKH * BL) + j * BL:
                                     rp * (NKH * BL) + (j + 1) * BL]
                        for g in range(4):
                            last = (j == NKH - 1 and g == 3)
                            ins = tensor.matmul(
                                zA[GOFF[g]:GOFF[g] + BL, :],
                                lhsT,
                                wh_sb[:, j * 4096 + g * NT:j * 4096 + (g + 1) * NT],
                                start=False, stop=last,
                                tile_position=(0, GOFF[g]), skip_group_check=True,
                            )
                        if last:
                            ins.then_inc(mm_sem, 1)
                    # bank B z
                    if t >= 1:
                        tensor.wait_ge(zfree_sem, 2 * (t - 1) + 2)
                    for g in range(4):
                        tensor.matmul(
                            zB[GOFF[g]:GOFF[g] + BL, :],
                            E_sb[:, u * BL:(u + 1) * BL],
                            XZw[:, (w % 2) * 4096 + (4 + g) * NT:(w % 2) * 4096 + (5 + g) * NT],
                            start=True, stop=False,
                            tile_position=(0, GOFF[g]), skip_group_check=True,
                        )
                    for j in range(NKH):
                        lhsT = hT_sb[:, rp * (NKH * BL) + j * BL:
                                     rp * (NKH * BL) + (j + 1) * BL]
                        for g in range(4):
                            last = (j == NKH - 1 and g == 3)
                            ins = tensor.matmul(
                                zB[GOFF[g]:GOFF[g] + BL, :],
                                lhsT,
                                wh_sb[:, j * 4096 + (4 + g) * NT:j * 4096 + (5 + g) * NT],
                                start=False, stop=last,
                                tile_position=(0, GOFF[g]), skip_group_check=True,
                            )
                        if last:
                            ins.then_inc(mm_sem, 1)
                    # interleaved next-window xz slab matmuls (even local steps)
                    if u % 2 == 0 and w + 1 < nwin:
                        s = u // 2
                        if s >= 2:
                            # psum slab bank reuse: slab s-2 of this window
                            # must be copied out first
                            tensor.wait_ge(xzc_sem, NSLAB * (w + 1) + s - 1)
                        slab_mms(tensor, w + 1, s, True)

            @block.scalar
            def _(scalar):
                scalar.wait_ge(dma_sem, 16 * NDMA)
                # window 0 slab copies
                for s in range(NSLAB):
                    scalar.wait_ge(slab_sem, s + 1)
                    nc.scalar.copy(
                        XZw[:, s * NT:(s + 1) * NT], slab_ps[s % 2][:, :],
                    ).then_inc(xzc_sem, 1)

                for t in range(t_steps):
                    w, u = t // WSTEP, t % WSTEP
                    wp = (t - 1) % 2  # hT parity written (for h(t-1))
                    if t >= 1:
                        scalar.wait_ge(tr_sem, 2 * (t - 1) + 1)
                        nc.scalar.copy(
                            hT_sb[:, wp * (NKH * BL):wp * (NKH * BL) + 4 * BL],
                            trbuf[:, 0:4 * BL],
                        ).then_inc(cp_sem, 1)
                        scalar.wait_ge(tr_sem, 2 * (t - 1) + 2)
                        nc.scalar.copy(
                            hT_sb[:, wp * (NKH * BL) + 4 * BL:wp * (NKH * BL) + 8 * BL],
                            trbuf[:, 4 * BL:8 * BL],
                        ).then_inc(cp_sem, 1)
                    scalar.wait_ge(mm_sem, 2 * t + 1)
                    if t >= 1:
                        scalar.wait_ge(h_sem, 2 * (t - 1) + 1)
                    nc.scalar.activation(
                        s_sb[:, 0:512], zA[:, :], AF.Sigmoid,
                    ).then_inc(sig_sem, 1)
                    nc.scalar.activation(
                        g_sb[:, 0:512], zA[96:96 + BL, :], AF.Tanh,
                    ).then_inc(zfree_sem, 1)
                    scalar.wait_ge(csum_sem, 2 * t + 1)
                    nc.scalar.activation(
                        tc_sb[64:64 + BL, 0:512], c_sb[32:32 + BL, 0:512], AF.Tanh,
                    ).then_inc(tanh_sem, 1)
                    scalar.wait_ge(mm_sem, 2 * t + 2)
                    if t >= 1:
                        scalar.wait_ge(h_sem, 2 * (t - 1) + 2)
                    nc.scalar.activation(
                        s_sb[:, 512:1024], zB[:, :], AF.Sigmoid,
                    ).then_inc(sig_sem, 1)
                    nc.scalar.activation(
                        g_sb[:, 512:1024], zB[96:96 + BL, :], AF.Tanh,
                    ).then_inc(zfree_sem, 1)
                    scalar.wait_ge(csum_sem, 2 * t + 2)
                    nc.scalar.activation(
                        tc_sb[64:64 + BL, 512:1024], c_sb[32:32 + BL, 512:1024], AF.Tanh,
                    ).then_inc(tanh_sem, 1)
                    # next-window slab copy (odd local steps)
                    if u % 2 == 1 and w + 1 < nwin:
                        s = u // 2
                        scalar.wait_ge(slab_sem, NSLAB * (w + 1) + s + 1)
                        nc.scalar.copy(
                            XZw[:, ((w + 1) % 2) * 4096 + s * NT:
                                ((w + 1) % 2) * 4096 + (s + 1) * NT],
                            slab_ps[s % 2][:, :],
                        ).then_inc(xzc_sem, 1)

            @block.vector
            def _(vector):
                for t in range(t_steps):
                    for bk in range(2):
                        lo = bk * 512
                        cols = slice(lo, lo + 512)
                        vector.wait_ge(sig_sem, 2 * t + bk + 1)
                        nc.vector.tensor_mul(
                            c1_sb[32:32 + BL, cols], s_sb[32:32 + BL, cols],
                            c_sb[32:32 + BL, cols],
                        )
                        vector.wait_ge(zfree_sem, 2 * t + bk + 1)
                        nc.vector.tensor_mul(
                            t1_sb[32:32 + BL, cols], s_sb[0:BL, cols], g_sb[0:BL, cols],
                        )
                        # drain: csum reads c1/t1 written just above (same-engine
                        # RAW); also orders last step's csum ahead of this c1
                        vector.drain()
                        nc.vector.tensor_add(
                            c_sb[32:32 + BL, cols],
                            c1_sb[32:32 + BL, cols], t1_sb[32:32 + BL, cols],
                        ).then_inc(csum_sem, 1)
                        vector.wait_ge(tanh_sem, 2 * t + bk + 1)
                        if t >= 1:
                            # WAR: h(t) overwrites h_sb read by transposes of h(t-1)
                            vector.wait_ge(tr_sem, 2 * (t - 1) + bk + 1)
                        if t < t_steps - 1:
                            nc.vector.tensor_mul(
                                h_sb[:, cols], s_sb[64:64 + BL, cols],
                                tc_sb[64:64 + BL, cols],
                            ).then_inc(h_sem, 1)
                        else:
                            nc.vector.tensor_mul(
                                hlast[0:BL, cols], s_sb[64:64 + BL, cols],
                                tc_sb[64:64 + BL, cols],
                            ).then_inc(h_sem, 1)

    return nc


def _prep_inputs(x, Wx, Wh, b):
    """Host-side layout prep (pure layout/dtype, no compute)."""
    f16 = np.float16
    t_steps = x.shape[1]
    # [Wx; Wh; b] rows, columns permuted to per-bank [i|f|o|g] blocks.
    Wfull = np.concatenate([Wx, Wh, b[None, :]], axis=0).astype(np.float32)
    cols = []
    for bank in range(2):
        u0, u1 = bank * NT, (bank + 1) * NT
        cols.append(np.arange(0 * U + u0, 0 * U + u1))       # i
        cols.append(np.arange(1 * U + u0, 1 * U + u1))       # f
        cols.append(np.arange(3 * U + u0, 3 * U + u1))       # o
        cols.append(np.arange(2 * U + u0, 2 * U + u1))       # g
    perm = np.concatenate(cols)
    Wp = np.ascontiguousarray(Wfull[:, perm]).astype(f16)

    # per-core xT: [kc, p, t*BL + b] = x[core*BL+b, t, kc*128+p]
    xqs = []
    for core in range(NCORES):
        xs = x[core * BL:(core + 1) * BL].astype(np.float32)      # [BL, T, D]
        xt = np.ascontiguousarray(np.transpose(xs, (2, 1, 0)))    # [D, T, BL]
        xt = xt.reshape(NKX, 128, t_steps * BL)
        xqs.append(np.ascontiguousarray(xt).astype(f16))

    E = np.zeros((128, WSTEP * BL), dtype=f16)
    for tl in range(WSTEP):
        for bb in range(BL):
            E[tl * BL + bb, tl * BL + bb] = 1.0
    mq = np.zeros((BL + 1, 128), dtype=f16)
    for i_ in range(BL):
        mq[i_, i_] = 1.0
    mq[BL, :] = 1.0
    zzero = np.zeros((128, 1024), dtype=f16)
    return Wp, xqs, E, mq, zzero


def kernel(x, Wx, Wh, b):
    x = np.asarray(x, dtype=np.float32)
    Wx = np.asarray(Wx, dtype=np.float32)
    Wh = np.asarray(Wh, dtype=np.float32)
    b = np.asarray(b, dtype=np.float32)
    t_steps = x.shape[1]

    Wp, xqs, E, mq, zzero = _prep_inputs(x, Wx, Wh, b)
    nc = _build_nc(t_steps)

    from concourse.bass_utils import run_bass_kernel_spmd
    core_ids = list(range(NCORES))
    in_maps = [{"wq": Wp, "xq": xqs[i], "eq": E, "mq": mq, "zz": zzero}
               for i in core_ids]
    res = run_bass_kernel_spmd(nc, in_maps, core_ids,
                               trace=bool(globals().get("TRACE", False)))
    globals()["LAST_EXEC_NS"] = res.exec_time_ns

    h_parts = [res.results[i]["out"].astype(np.float32) for i in core_ids]
    return np.concatenate(h_parts, axis=0)
